# revision 2
# baseline (speedup 1.0000x reference)
"""Trainium2 Bass kernel v2 for nn_ButterflyRotation (B=8192, D=4096, L=12).

Same two-stage matmul factorization as v1 (see kernel.py docstring), with
engine assignment and data delivery rebuilt around HW microbenchmarks:

  - PSUM->SBUF evac copies all go to ACT: PSUM-src copies measured
    ~0.33us per [128,1024] on HW on both ACT and DVE, stride-insensitive
    (vs the ~1us the cost model charges), so ACT absorbs all 64 evacs
    (~22us/slab) while DVE runs only the 8 stream transposes (~29us) --
    the measured critical engine. Single-shot 1-core HW: ~33us vs 68us
    for v1 (ABBA wall-clock slope, outer-replicated program).
  - Weights ship as two contiguous [128,4096] bf16 images (wa_img/wb_img,
    host-packed, zero-padded block diagonal for wb) on the Activation
    HWDGE ring, overlapping the x loads on the SP ring. No gpsimd SWDGE,
    no memset, no identity tensor.
  - Output DMA rides the Activation HWDGE ring (free after the 2 MiB of
    weights), input DMA the SP ring.
  - PE warmup matmuls run on a tiny memset tile to ramp the PE pstate
    during the DMA head.
"""

from contextlib import ExitStack

import numpy as np
import ml_dtypes

import concourse.bass as bass  # noqa: F401
import concourse.tile as tile
from concourse import bacc, mybir
from concourse import bass_utils

F32 = mybir.dt.float32
BF16 = mybir.dt.bfloat16
NP_BF16 = ml_dtypes.bfloat16

DIM = 4096
LAYERS = 12
BATCH = 8192
N_CORES = 8
BC = BATCH // N_CORES          # 1024 batch rows per core
GROUP = 256                    # batch rows per pipeline group
NGRP = BC // GROUP             # 4
NB_O = 32                      # outer blocks j[11:7]
NQ = 128                       # inner j[6:0]

_cache = {}


# ---------------------------------------------------------------- host math
def _apply_layers(x, angles, layers):
    B, d = x.shape
    out = x
    for l in layers:
        stride = 1 << l
        nb = d // (2 * stride)
        theta = angles[l].reshape(nb, stride)
        c = np.cos(theta)
        s = np.sin(theta)
        o = out.reshape(B, nb, 2, stride)
        xl = o[:, :, 0, :]
        xr = o[:, :, 1, :]
        new_l = c * xl + s * xr
        new_r = -s * xl + c * xr
        out = np.stack([new_l, new_r], axis=2).reshape(B, d)
    return out


def _build_weights(angles):
    """wa_img[q, o*128+q'] = (T1^T)[o*128+q, o*128+q'] (stage-A lhsT).
    wb_img[p, v*128+m]: stage-B lhsT image, block-diagonal in j65."""
    a64 = angles.astype(np.float64)
    I = np.eye(DIM, dtype=np.float64)
    M1 = _apply_layers(I, a64, range(0, 7))     # T1^T, block diagonal
    M2 = _apply_layers(I, a64, range(7, 12))    # T2^T, q-diagonal

    wa_img = np.zeros((NQ, NB_O * NQ), dtype=NP_BF16)
    for o in range(NB_O):
        wa_img[:, o*128:(o+1)*128] = M1[o*128:(o+1)*128,
                                        o*128:(o+1)*128].astype(NP_BF16)

    wb_img = np.zeros((NQ, 32 * NQ), dtype=NP_BF16)
    for j65 in range(4):
        for v in range(32):
            q = j65 * 32 + v
            # 32x32 block at partitions j65*32.., columns v*128 + j65*32..
            wb_img[j65*32:(j65+1)*32, v*128 + j65*32:v*128 + (j65+1)*32] = \
                M2[q::128, q::128].astype(NP_BF16)
    return wa_img, wb_img


def _prep_x(x_core):
    """bf16 + transpose a (BC, DIM) fp32 slab to [g][q][o*GROUP+b]."""
    xb = x_core.astype(NP_BF16).view(np.uint16)
    xb = xb.reshape(NGRP, GROUP, NB_O, NQ).transpose(0, 3, 2, 1)
    return np.ascontiguousarray(xb).view(NP_BF16).reshape(NGRP, NQ, NB_O * GROUP)


# ---------------------------------------------------------------- device IR
# Evac engine split: per group, which of the 8 stage-A / 8 stage-B
# PSUM-evac copies go to DVE (rest go to ACT).
DVE_P2 = (0, 0, 0, 0)
DVE_P4 = (0, 0, 0, 0)


def _build_program(reps=1, dve_p2=None, dve_p4=None, p2w=4, p4w=8,
                   z1_bufs=3, pa_bufs=2, pb_bufs=2, outer=1):
    dve_p2 = DVE_P2 if dve_p2 is None else dve_p2
    dve_p4 = DVE_P4 if dve_p4 is None else dve_p4
    if isinstance(dve_p2, int):
        dve_p2 = (dve_p2,) * NGRP
    if isinstance(dve_p4, int):
        dve_p4 = (dve_p4,) * NGRP
    np2 = NB_O // p2w           # stage-A evac copies per group
    nvh = 32 // p4w             # stage-B evac copies per c-chunk
    nc = bacc.Bacc("TRN2", target_bir_lowering=False, debug=False,
                   num_devices=N_CORES)
    x_d = nc.dram_tensor("xt", [NGRP, NQ, NB_O * GROUP], BF16,
                         kind="ExternalInput").ap()
    wa_d = nc.dram_tensor("wa", [NQ, NB_O * NQ], BF16,
                          kind="ExternalInput").ap()
    wb_d = nc.dram_tensor("wb", [NQ, 32 * NQ], BF16,
                          kind="ExternalInput").ap()
    out_d = nc.dram_tensor("out", [BC, DIM], BF16, kind="ExternalOutput").ap()

    with tile.TileContext(nc, trace_sim=False) as tc:
      for it in range(outer):
       with ExitStack() as ctx:
        wpool = ctx.enter_context(tc.tile_pool(name=f"w{it}", bufs=1))
        z1pool = ctx.enter_context(tc.tile_pool(name=f"z1{it}", bufs=z1_bufs))
        z2pool = ctx.enter_context(tc.tile_pool(name=f"z2{it}", bufs=3))
        z3pool = ctx.enter_context(tc.tile_pool(name=f"z3{it}", bufs=3))
        opool = ctx.enter_context(tc.tile_pool(name=f"xout{it}", bufs=3))
        pa = ctx.enter_context(tc.tile_pool(name=f"pa{it}", bufs=pa_bufs,
                                            space="PSUM"))
        pb = ctx.enter_context(tc.tile_pool(name=f"pb{it}", bufs=pb_bufs,
                                            space="PSUM"))

        wa_sb = wpool.tile([128, NB_O * 128], BF16, tag=f"wa{it}")
        wb_sb = wpool.tile([128, 32 * 128], BF16, tag=f"wb{it}")
        warm = wpool.tile([128, 64], BF16, tag=f"warm{it}")
        # weights ride the ACT HWDGE ring (the out-DMA ring, idle at start)
        # so they overlap the first x loads on the SP ring; wa in halves so
        # stage A's first matmuls gate on 512 KiB only.
        # wa on the SP ring ahead of the x loads (stage A gates on it);
        # wb on the ACT ring (idle until the first out-DMA ~15us in).
        # Balances the two rings at 9 MiB each instead of 8/10.
        nc.sync.dma_start(wa_sb[:, :2048], wa_d[:, :2048])
        nc.sync.dma_start(wa_sb[:, 2048:], wa_d[:, 2048:])
        nc.scalar.dma_start(wb_sb[:], wb_d[:])

        # PE pstate warm-up during the DMA head
        nc.gpsimd.memset(warm[:], 0.0)
        for i in range(16):
            pw = pa.tile([64, 64], F32, tag=f"pa{it}", name=f"warm_{it}_{i}")
            nc.tensor.matmul(pw[:], warm[:, :64], warm[:])

        for g in [g for _ in range(reps) for g in range(NGRP)]:
            z1 = z1pool.tile([128, NB_O * GROUP], BF16, tag=f"z1{it}")  # [q,(o,b)]
            z2 = z2pool.tile([128, NB_O * GROUP], BF16, tag=f"z2{it}")  # [q,(b,o)]
            z3 = z3pool.tile([128, NB_O * GROUP], BF16, tag=f"z3{it}")  # [p,(b,v)]

            # --- phase 1: load (already d-major from host prep) -----------
            for p in range(4):
                nc.sync.dma_start(z1[:, p*2048:(p+1)*2048],
                                  x_d[g, :, p*2048:(p+1)*2048])

            # --- phase 2: stage A matmuls, p2w o-blocks per PSUM tile -----
            for oq in range(np2):
                ps_a = pa.tile([128, p2w * GROUP], F32, tag=f"pa{it}")
                for oo in range(p2w):
                    o = oq * p2w + oo
                    nc.tensor.matmul(ps_a[:, oo*GROUP:(oo+1)*GROUP],
                                     wa_sb[:, o*128:(o+1)*128],
                                     z1[:, o*GROUP:(o+1)*GROUP])
                # evac: Z2 free = b*32 + o (strided dst; PSUM-src copies are
                # stride-insensitive on both DVE and ACT per microbench)
                dst = z2[:].rearrange("q (b o) -> q b o", o=32)[
                    :, :, oq*p2w:(oq+1)*p2w]
                src = ps_a[:].rearrange("q (o b) -> q b o", b=GROUP)
                n2 = dve_p2[g]
                if n2 and (oq * n2) % np2 < n2:
                    nc.vector.tensor_copy(dst, src)
                else:
                    nc.scalar.copy(dst, src)

            # --- phase 3: 32x32 stream transpose (v<->o), contig input ----
            for c in range(2):
                sl = slice(c * 4096, (c + 1) * 4096)
                nc.vector.transpose(z3[:, sl], z2[:, sl])

            # --- phase 4: stage B matmuls (lhsT = data) -------------------
            z3v = z3[:].rearrange("p (b v) -> p b v", v=32)
            for c in range(2):
                xo = opool.tile([128, DIM], BF16, tag=f"xo{it}",
                                name=f"xo_{it}_{g}_{c}")
                for vh in range(nvh):
                    ps_b = pb.tile([128, p4w * 128], F32, tag=f"pb{it}")
                    for vv in range(p4w):
                        v = vh * p4w + vv
                        lhsT = z3v[:, c*128:(c+1)*128, v]    # [p, b] strided
                        nc.tensor.matmul(ps_b[:, vv*128:(vv+1)*128],
                                         lhsT,
                                         wb_sb[:, v*128:(v+1)*128])
                    # evac scatter: out free j' = o'*128 + j65*32 + v
                    dst = xo[:].rearrange(
                        "b (o f v) -> b o f v", f=4, v=32)[
                        :, :, :, vh*p4w:(vh+1)*p4w]
                    src = ps_b[:].rearrange("b (v f o) -> b o f v",
                                            v=p4w, f=4)
                    idx = c * nvh + vh
                    n4 = dve_p4[g]
                    if n4 and (idx * n4) % (2 * nvh) < n4:
                        nc.vector.tensor_copy(dst, src)
                    else:
                        nc.scalar.copy(dst, src)
                # --- phase 5: store on the ACT ring -----------------------
                row0 = g * GROUP + c * 128
                nc.scalar.dma_start(out_d[row0:row0 + 128, :], xo[:])

    nc.compile()
    return nc


def _get_program():
    if "nc" not in _cache:
        _cache["nc"] = _build_program()
    return _cache["nc"]


# ---------------------------------------------------------------- entry
def kernel(x, angles):
    x = np.ascontiguousarray(np.asarray(x, dtype=np.float32))
    angles = np.asarray(angles, dtype=np.float32)
    assert x.shape == (BATCH, DIM) and angles.shape == (LAYERS, DIM // 2)

    WA, WB = _build_weights(angles)
    nc = _get_program()

    in_maps = []
    for core in range(N_CORES):
        in_maps.append({
            "xt": _prep_x(x[core * BC:(core + 1) * BC]),
            "wa": WA, "wb": WB,
        })
    res = bass_utils.run_bass_kernel_spmd(
        nc, in_maps, core_ids=list(range(N_CORES)))
    out = np.concatenate([r["out"].astype(np.float32) for r in res.results],
                         axis=0)
    return out


# revision 4
# speedup vs baseline: 1.2746x; 1.2746x over previous
"""Trainium2 Bass kernel v2 for nn_ButterflyRotation (B=8192, D=4096, L=12).

Same two-stage matmul factorization as v1 (see kernel.py docstring), with
engine assignment and data delivery rebuilt around HW microbenchmarks:

  - PSUM->SBUF evac copies all go to ACT: PSUM-src copies measured
    ~0.33us per [128,1024] on HW on both ACT and DVE, stride-insensitive
    (vs the ~1us the cost model charges), so ACT absorbs all 64 evacs
    (~22us/slab) while DVE runs only the 8 stream transposes (~29us) --
    the measured critical engine. Single-shot 1-core HW: ~33us vs 68us
    for v1 (ABBA wall-clock slope, outer-replicated program).
  - Weights ship as two contiguous [128,4096] bf16 images (wa_img/wb_img,
    host-packed, zero-padded block diagonal for wb) on the Activation
    HWDGE ring, overlapping the x loads on the SP ring. No gpsimd SWDGE,
    no memset, no identity tensor.
  - Output DMA rides the Activation HWDGE ring (free after the 2 MiB of
    weights), input DMA the SP ring.
  - PE warmup matmuls run on a tiny memset tile to ramp the PE pstate
    during the DMA head.
"""

from contextlib import ExitStack

import numpy as np
import ml_dtypes

import concourse.bass as bass  # noqa: F401
import concourse.tile as tile
from concourse import bacc, mybir
from concourse import bass_utils

F32 = mybir.dt.float32
BF16 = mybir.dt.bfloat16
NP_BF16 = ml_dtypes.bfloat16

DIM = 4096
LAYERS = 12
BATCH = 8192
N_CORES = 8
BC = BATCH // N_CORES          # 1024 batch rows per core
GROUP = 256                    # batch rows per pipeline group
NGRP = BC // GROUP             # 4
NB_O = 32                      # outer blocks j[11:7]
NQ = 128                       # inner j[6:0]

_cache = {}


# ---------------------------------------------------------------- host math
def _apply_layers(x, angles, layers):
    B, d = x.shape
    out = x
    for l in layers:
        stride = 1 << l
        nb = d // (2 * stride)
        theta = angles[l].reshape(nb, stride)
        c = np.cos(theta)
        s = np.sin(theta)
        o = out.reshape(B, nb, 2, stride)
        xl = o[:, :, 0, :]
        xr = o[:, :, 1, :]
        new_l = c * xl + s * xr
        new_r = -s * xl + c * xr
        out = np.stack([new_l, new_r], axis=2).reshape(B, d)
    return out


def _build_weights(angles):
    """wa_img[q, o*128+q'] = (T1^T)[o*128+q, o*128+q'] (stage-A lhsT).
    wb_img[p, v*128+m]: stage-B lhsT image, block-diagonal in j65."""
    a64 = angles.astype(np.float64)
    I = np.eye(DIM, dtype=np.float64)
    M1 = _apply_layers(I, a64, range(0, 7))     # T1^T, block diagonal
    M2 = _apply_layers(I, a64, range(7, 12))    # T2^T, q-diagonal

    wa_img = np.zeros((NQ, NB_O * NQ), dtype=NP_BF16)
    for o in range(NB_O):
        wa_img[:, o*128:(o+1)*128] = M1[o*128:(o+1)*128,
                                        o*128:(o+1)*128].astype(NP_BF16)

    wb_img = np.zeros((NQ, 32 * NQ), dtype=NP_BF16)
    for j65 in range(4):
        for v in range(32):
            q = j65 * 32 + v
            # 32x32 block at partitions j65*32.., columns v*128 + j65*32..
            wb_img[j65*32:(j65+1)*32, v*128 + j65*32:v*128 + (j65+1)*32] = \
                M2[q::128, q::128].astype(NP_BF16)
    return wa_img, wb_img


def _prep_x(x_core):
    """bf16 + transpose a (BC, DIM) fp32 slab to [g][q][o*GROUP+b]."""
    xb = x_core.astype(NP_BF16).view(np.uint16)
    xb = xb.reshape(NGRP, GROUP, NB_O, NQ).transpose(0, 3, 2, 1)
    return np.ascontiguousarray(xb).view(NP_BF16).reshape(NGRP, NQ, NB_O * GROUP)


# ---------------------------------------------------------------- device IR
# Evac engine split: per group, which of the 8 stage-A / 8 stage-B
# PSUM-evac copies go to DVE (rest go to ACT).
DVE_P2 = (0, 0, 0, 0)
DVE_P4 = (0, 0, 0, 0)


def _build_program(reps=1, dve_p2=None, dve_p4=None, p2w=4, p4w=8,
                   z1_bufs=3, pa_bufs=2, pb_bufs=2, outer=1):
    dve_p2 = DVE_P2 if dve_p2 is None else dve_p2
    dve_p4 = DVE_P4 if dve_p4 is None else dve_p4
    if isinstance(dve_p2, int):
        dve_p2 = (dve_p2,) * NGRP
    if isinstance(dve_p4, int):
        dve_p4 = (dve_p4,) * NGRP
    np2 = NB_O // p2w           # stage-A evac copies per group
    nvh = 32 // p4w             # stage-B evac copies per c-chunk
    nc = bacc.Bacc("TRN2", target_bir_lowering=False, debug=False,
                   num_devices=N_CORES)
    x_d = nc.dram_tensor("xt", [NGRP, NQ, NB_O * GROUP], BF16,
                         kind="ExternalInput").ap()
    wa_d = nc.dram_tensor("wa", [NQ, NB_O * NQ], BF16,
                          kind="ExternalInput").ap()
    wb_d = nc.dram_tensor("wb", [NQ, 32 * NQ], BF16,
                          kind="ExternalInput").ap()
    out_d = nc.dram_tensor("out", [BC, DIM], BF16, kind="ExternalOutput").ap()

    with tile.TileContext(nc, trace_sim=False) as tc:
      for it in range(outer):
       with ExitStack() as ctx:
        wpool = ctx.enter_context(tc.tile_pool(name=f"w{it}", bufs=1))
        z1pool = ctx.enter_context(tc.tile_pool(name=f"z1{it}", bufs=z1_bufs))
        z2pool = ctx.enter_context(tc.tile_pool(name=f"z2{it}", bufs=3))
        z3pool = ctx.enter_context(tc.tile_pool(name=f"z3{it}", bufs=3))
        opool = ctx.enter_context(tc.tile_pool(name=f"xout{it}", bufs=3))
        pa = ctx.enter_context(tc.tile_pool(name=f"pa{it}", bufs=pa_bufs,
                                            space="PSUM"))
        pb = ctx.enter_context(tc.tile_pool(name=f"pb{it}", bufs=pb_bufs,
                                            space="PSUM"))

        wa_sb = wpool.tile([128, NB_O * 128], BF16, tag=f"wa{it}")
        wb_sb = wpool.tile([128, 32 * 128], BF16, tag=f"wb{it}")
        warm = wpool.tile([128, 64], BF16, tag=f"warm{it}")
        # weights ride the ACT HWDGE ring (the out-DMA ring, idle at start)
        # so they overlap the first x loads on the SP ring; wa in halves so
        # stage A's first matmuls gate on 512 KiB only.
        # wa on the SP ring ahead of the x loads (stage A gates on it);
        # wb on the ACT ring (idle until the first out-DMA ~15us in).
        # Balances the two rings at 9 MiB each instead of 8/10.
        nc.sync.dma_start(wa_sb[:, :2048], wa_d[:, :2048])
        nc.sync.dma_start(wa_sb[:, 2048:], wa_d[:, 2048:])
        nc.scalar.dma_start(wb_sb[:], wb_d[:])

        # PE pstate warm-up during the DMA head
        nc.gpsimd.memset(warm[:], 0.0)
        for i in range(16):
            pw = pa.tile([64, 64], F32, tag=f"pa{it}", name=f"warm_{it}_{i}")
            nc.tensor.matmul(pw[:], warm[:, :64], warm[:])

        for g in [g for _ in range(reps) for g in range(NGRP)]:
            z1 = z1pool.tile([128, NB_O * GROUP], BF16, tag=f"z1{it}")  # [q,(o,b)]
            z2 = z2pool.tile([128, NB_O * GROUP], BF16, tag=f"z2{it}")  # [q,(b,o)]
            z3 = z3pool.tile([128, NB_O * GROUP], BF16, tag=f"z3{it}")  # [p,(b,v)]

            # --- phase 1: load (already d-major from host prep) -----------
            for p in range(4):
                nc.sync.dma_start(z1[:, p*2048:(p+1)*2048],
                                  x_d[g, :, p*2048:(p+1)*2048])

            # --- phase 2: stage A matmuls, p2w o-blocks per PSUM tile -----
            for oq in range(np2):
                ps_a = pa.tile([128, p2w * GROUP], F32, tag=f"pa{it}")
                for oo in range(p2w):
                    o = oq * p2w + oo
                    nc.tensor.matmul(ps_a[:, oo*GROUP:(oo+1)*GROUP],
                                     wa_sb[:, o*128:(o+1)*128],
                                     z1[:, o*GROUP:(o+1)*GROUP])
                # evac: Z2 free = b*32 + o (strided dst; PSUM-src copies are
                # stride-insensitive on both DVE and ACT per microbench)
                dst = z2[:].rearrange("q (b o) -> q b o", o=32)[
                    :, :, oq*p2w:(oq+1)*p2w]
                src = ps_a[:].rearrange("q (o b) -> q b o", b=GROUP)
                n2 = dve_p2[g]
                if n2 and (oq * n2) % np2 < n2:
                    nc.vector.tensor_copy(dst, src)
                else:
                    nc.scalar.copy(dst, src)

            # --- phase 3: 32x32 stream transpose (v<->o), contig input ----
            for c in range(2):
                sl = slice(c * 4096, (c + 1) * 4096)
                nc.vector.transpose(z3[:, sl], z2[:, sl])

            # --- phase 4: stage B matmuls (lhsT = data) -------------------
            z3v = z3[:].rearrange("p (b v) -> p b v", v=32)
            for c in range(2):
                xo = opool.tile([128, DIM], BF16, tag=f"xo{it}",
                                name=f"xo_{it}_{g}_{c}")
                for vh in range(nvh):
                    ps_b = pb.tile([128, p4w * 128], F32, tag=f"pb{it}")
                    for vv in range(p4w):
                        v = vh * p4w + vv
                        lhsT = z3v[:, c*128:(c+1)*128, v]    # [p, b] strided
                        nc.tensor.matmul(ps_b[:, vv*128:(vv+1)*128],
                                         lhsT,
                                         wb_sb[:, v*128:(v+1)*128])
                    # evac scatter: out free j' = o'*128 + j65*32 + v
                    dst = xo[:].rearrange(
                        "b (o f v) -> b o f v", f=4, v=32)[
                        :, :, :, vh*p4w:(vh+1)*p4w]
                    src = ps_b[:].rearrange("b (v f o) -> b o f v",
                                            v=p4w, f=4)
                    idx = c * nvh + vh
                    n4 = dve_p4[g]
                    if n4 and (idx * n4) % (2 * nvh) < n4:
                        nc.vector.tensor_copy(dst, src)
                    else:
                        nc.scalar.copy(dst, src)
                # --- phase 5: store on the ACT ring -----------------------
                row0 = g * GROUP + c * 128
                nc.scalar.dma_start(out_d[row0:row0 + 128, :], xo[:])

    nc.compile()
    return nc


def _get_program():
    if "nc" not in _cache:
        _cache["nc"] = _build_program()
    return _cache["nc"]


# ---------------------------------------------------------------- entry
def kernel(x, angles):
    x = np.ascontiguousarray(np.asarray(x, dtype=np.float32))
    angles = np.asarray(angles, dtype=np.float32)
    assert x.shape == (BATCH, DIM) and angles.shape == (LAYERS, DIM // 2)

    WA, WB = _build_weights(angles)
    nc = _get_program()

    in_maps = []
    for core in range(N_CORES):
        in_maps.append({
            "xt": _prep_x(x[core * BC:(core + 1) * BC]),
            "wa": WA, "wb": WB,
        })
    res = bass_utils.run_bass_kernel_spmd(
        nc, in_maps, core_ids=list(range(N_CORES)))
    out = np.concatenate([r["out"].astype(np.float32) for r in res.results],
                         axis=0)
    return out
